# revision 17
# baseline (speedup 1.0000x reference)
"""TRN2 Bass kernel for nn_MGCN (MultiChev GCN forward, N=4096 nodes, F=200).

Sharding: 8-way row-shard over the node axis (512 rows/core). Each core
recomputes the full attrT = sigmoid(ggl_w.T @ x.T + b) (bf16, spilled to a
DRAM scratch) so the gram matmul attr@attr.T needs no large collective; the
per-row reductions (row-max, degree) and tx1 are exchanged with three small
collectives (AllGather rmax||dinv, AllGather tx1, AllReduce BN stats).

Algebraic identities used (validated against the reference in fp32/bf16
numpy): the cosine matrix is symmetric with diagonal exactly 1, so
  M[i,j] = -dinv_i*dinv_j*cos[i,j]/rmax_j        (without zeroing the diag)
plus a per-row rank-1 correction  tx += dinv_i^2*(1/rmax_i) * x_i  replaces
the self-loop removal. BatchNorm biases b1/b2/b3 cancel inside training-mode
BN (and are exactly zero for this problem's inputs), so they are skipped.

Precision: big matmuls (attrT, gram, M@x, M@tx1) in bf16 with fp32
accumulation; the small s = tx @ w projections in fp32 (they dominate the
output error otherwise); all normalization arithmetic in fp32.
"""
import sys

sys.path.insert(0, "/opt/trn_rl_repo")

import numpy as np

import concourse.bass as bass
import concourse.mybir as mybir
import concourse.tile as tile
from concourse import bacc
from concourse import bass_utils
from concourse.masks import make_identity

NCORE = 8
N = 4096
F = 200
O = 200
S = N // NCORE          # 512 rows per core
P = 128
NT = N // P             # 32 tiles of 128 over the node axis
MT_ = S // P            # 4 row-chunks per shard
F0, F1 = 128, F - 128   # feature split 128 + 72
EPS_COS = 1e-8
EPS_BN = 1e-5

FP = mybir.dt.float32
BF = mybir.dt.bfloat16
AX = mybir.AxisListType
OP = mybir.AluOpType
AF = mybir.ActivationFunctionType

# Build variants for perf attribution (test-only; harness uses defaults).
BUILD_STAGE = "full"      # "A" | "N" | "B" | "full"
USE_COLLECTIVES = True
NUM_CC = 3


def build_nc():
    nc = bacc.Bacc("TRN2", target_bir_lowering=False, debug=False,
                   enable_asserts=False, num_devices=NCORE)

    # ---- kernel I/O ----
    x = nc.dram_tensor("x", [N, F], FP, kind="ExternalInput")
    xT = nc.dram_tensor("xt", [F, N], FP, kind="ExternalInput")
    xs = nc.dram_tensor("xs", [S, F], FP, kind="ExternalInput")
    xsT = nc.dram_tensor("xst", [F, S], FP, kind="ExternalInput")
    gw = nc.dram_tensor("gw", [F, N], FP, kind="ExternalInput")
    gb = nc.dram_tensor("gb", [N], FP, kind="ExternalInput")
    wproj = nc.dram_tensor("wproj", [6, F, O], FP, kind="ExternalInput")
    gamma = nc.dram_tensor("gamma", [O], FP, kind="ExternalInput")
    beta = nc.dram_tensor("beta", [O], FP, kind="ExternalInput")

    anorm = nc.dram_tensor("anorm", [S, N], FP, kind="ExternalOutput")
    xo = [nc.dram_tensor(f"xo{k}", [S, O], FP, kind="ExternalOutput")
          for k in range(3)]

    # ---- internal DRAM ----
    atd = nc.dram_tensor("atd", [N, N], BF)            # attrT scratch
    mtd = nc.dram_tensor("mtd", [N, S], BF)            # cosT scratch
    ccni = nc.dram_tensor("ccni", [1, S], FP)
    ccno = nc.dram_tensor("ccno", [NCORE, S], FP, addr_space="Shared")
    cc1i = nc.dram_tensor("cc1i", [2, S], FP)
    cc1o = nc.dram_tensor("cc1o", [2 * NCORE, S], FP, addr_space="Shared")
    cc2i = nc.dram_tensor("cc2i", [S, F], FP)
    cc2o = nc.dram_tensor("cc2o", [N, F], FP, addr_space="Shared")
    cc3i = nc.dram_tensor("cc3i", [6, O], FP)
    cc3o = nc.dram_tensor("cc3o", [6, O], FP, addr_space="Shared")

    with tile.TileContext(nc) as tc:
        with tc.tile_pool(name="pers", bufs=1) as pers:
            _build_body(nc, tc, pers, locals())
    nc.compile()
    return nc


def _build_body(nc, tc, pers, T):
    x, xT, xs, xsT, gw, gb = T["x"], T["xT"], T["xs"], T["xsT"], T["gw"], T["gb"]
    wproj, gamma, beta = T["wproj"], T["gamma"], T["beta"]
    anorm, xo = T["anorm"], T["xo"]
    atd, mtd = T["atd"], T["mtd"]
    cc1i, cc1o, cc2i, cc2o, cc3i, cc3o = (
        T["cc1i"], T["cc1o"], T["cc2i"], T["cc2o"], T["cc3i"], T["cc3o"])
    ccni, ccno = T["ccni"], T["ccno"]

    # ---------- small global persistent tiles ----------
    ninv_sb = pers.tile([P, S], FP, tag="ninv", name="ninv")
    ident = pers.tile([P, P], FP, tag="ident", name="ident")
    ones_bf = pers.tile([P, P], BF, tag="ones_bf", name="ones_bf")
    ones_f = pers.tile([P, P], FP, tag="ones_f", name="ones_f")
    rvec = pers.tile([P, 16], FP, tag="rvec", name="rvec")
    zero1 = pers.tile([P, 1], FP, tag="zero1", name="zero1")
    wj_sb = pers.tile([P, NT], FP, tag="wj", name="wj")
    dsh_bc = pers.tile([P, S], FP, tag="dsh_bc", name="dsh_bc")
    b_sb = pers.tile([P, NT], FP, tag="b_sb", name="b_sb")

    make_identity(nc, ident[:])
    nc.gpsimd.memset(ones_bf[:], 1.0)
    nc.gpsimd.memset(ones_f[:], 1.0)
    nc.gpsimd.memset(zero1[:], 0.0)

    with tc.tile_pool(name="persL", bufs=1) as persL:
        L_all = persL.tile([P, NT * S], BF, tag="L_all", name="L_all")

        # =============================================================
        # Phase A: attrT = sigmoid(gw.T @ xT + b) -> atd, L_all, ninv
        # =============================================================
        with tc.tile_pool(name="persA", bufs=1) as persA, \
             tc.tile_pool(name="pAst", bufs=1) as pAst, \
             tc.tile_pool(name="pAat", bufs=2) as pAat, \
             tc.tile_pool(name="pAsq", bufs=2) as pAsq, \
             tc.tile_pool(name="psA", bufs=2, space="PSUM") as psA, \
             tc.tile_pool(name="psAn", bufs=1, space="PSUM") as psAn, \
             tc.tile_pool(name="psA2", bufs=3, space="PSUM") as psA2:

            gw_bf = [persA.tile([P, N], BF, tag=f"gw_bf{i}", name=f"gw_bf{i}")
                     for i in range(2)]
            xt_bf = [persA.tile([P, N], BF, tag=f"xt_bf{i}", name=f"xt_bf{i}")
                     for i in range(2)]
            xs_bf = [persA.tile([P, S], BF, tag=f"xs_bf{i}", name=f"xs_bf{i}")
                     for i in range(2)]
            for i, (dst, src) in enumerate([(gw_bf[0], gw), (gw_bf[1], gw),
                                            (xt_bf[0], xT), (xt_bf[1], xT)]):
                lo, sz = (0, F0) if i % 2 == 0 else (F0, F1)
                st = pAst.tile([P, N], FP, tag="stageA", name="stageA")
                nc.sync.dma_start(st[:sz, :], src[lo:lo + sz, :])
                nc.vector.tensor_copy(dst[:sz, :], st[:sz, :])
            for i in range(2):
                lo, sz = (0, F0) if i == 0 else (F0, F1)
                st = pAst.tile([P, S], FP, tag="stageAs", name="stageAs")
                nc.sync.dma_start(st[:sz, :], xsT[lo:lo + sz, :])
                nc.vector.tensor_copy(xs_bf[i][:sz, :], st[:sz, :])

            nc.sync.dma_start(b_sb[:], gb.ap().rearrange("(t p) -> p t", p=P))

            ninv_ps = psAn.tile([P, S], FP, name="ninv_ps")
            fsz = [(0, F0), (F0, F1)]
            for kt in range(NT):
                kc = bass.ts(kt, P)
                pL = psA.tile([P, S], FP, tag="pL", name="pL")
                for fi, (lo, sz) in enumerate(fsz):
                    nc.tensor.matmul(pL[:], gw_bf[fi][:sz, kc],
                                     xs_bf[fi][:sz, :],
                                     start=(fi == 0), stop=(fi == 1))
                L_sl = L_all[:, kt * S:(kt + 1) * S]
                nc.scalar.activation(L_sl, pL[:], AF.Sigmoid,
                                     bias=b_sb[:, kt:kt + 1])
                sq = pAsq.tile([P, S], BF, tag="sqA", name="sqA")
                nc.vector.tensor_mul(sq[:], L_sl, L_sl)
                nc.tensor.matmul(ninv_ps[:], ones_bf[:], sq[:],
                                 start=(kt == 0), stop=(kt == NT - 1),
                                 skip_group_check=True)
                at = pAat.tile([P, N], BF, tag="at", name="at")
                for nb in range(8):
                    nbs = bass.ts(nb, 512)
                    pa = psA2.tile([P, 512], FP, tag="pa", name="pa")
                    for fi, (lo, sz) in enumerate(fsz):
                        nc.tensor.matmul(pa[:], gw_bf[fi][:sz, kc],
                                         xt_bf[fi][:sz, nbs],
                                         start=(fi == 0), stop=(fi == 1))
                    nc.scalar.activation(at[:, nbs], pa[:], AF.Sigmoid,
                                         bias=b_sb[:, kt:kt + 1])
                nc.sync.dma_start(atd[kt * P:(kt + 1) * P, :], at[:])
            nrow = pAst.tile([1, S], FP, tag="nrow", name="nrow")
            nc.vector.tensor_copy(nrow[:], ninv_ps[0:1, :])
            nc.sync.dma_start(ccni[0:1, :], nrow[:])
            nc.scalar.sqrt(ninv_sb[:], ninv_ps[:])
            nc.vector.tensor_scalar_max(ninv_sb[:], ninv_sb[:], EPS_COS)
            nc.vector.reciprocal(ninv_sb[:], ninv_sb[:])
        if USE_COLLECTIVES:
            nc.gpsimd.collective_compute(
                "AllGather", OP.bypass, replica_groups=[list(range(NCORE))],
                ins=[ccni.ap()], outs=[ccno.ap()])
        if BUILD_STAGE == "A":
            return

        with tc.tile_pool(name="persBig", bufs=1) as persBig:
            acc = [persBig.tile([P, N], FP, tag=f"acc{m}", name=f"acc{m}")
                   for m in range(MT_)]
            cinv_sb = persBig.tile([P, N], FP, tag="cinv", name="cinv")

            # =============================================================
            # Phase N: cinv from gathered per-shard norms (overlaps B)
            # =============================================================
            with tc.tile_pool(name="pN", bufs=1) as pN:
                crow = pN.tile([1, N], FP, tag="crow", name="crow")
                nc.sync.dma_start(crow[:], ccno.ap().rearrange(
                    "(a r) j -> a (r j)", a=1))
                nc.scalar.sqrt(crow[:], crow[:])
                nc.vector.tensor_scalar_max(crow[:], crow[:], EPS_COS)
                nc.vector.reciprocal(crow[:], crow[:])
                nc.gpsimd.partition_broadcast(cinv_sb[:], crow[:])
            if BUILD_STAGE == "N":
                return

            # =============================================================
            # Phase B: gram  acc[m] = (L.T @ attrT) over j-halves, k-octants
            # =============================================================
            with tc.tile_pool(name="pB", bufs=10) as pB, \
                 tc.tile_pool(name="psB", bufs=2, space="PSUM") as psB:
                JH = N // 2
                for jh in range(2):
                    for op_ in range(4):
                        rh = []
                        for ktl in range(8):
                            kt = op_ * 8 + ktl
                            t = pB.tile([P, JH], BF, tag="rhs", name="rhs")
                            nc.sync.dma_start(
                                t[:],
                                atd[kt * P:(kt + 1) * P,
                                    jh * JH:(jh + 1) * JH])
                            rh.append(t)
                        for m in range(MT_):
                            ps = psB.tile([P, JH], FP, tag="psB", name="psB")
                            for ktl in range(8):
                                kt = op_ * 8 + ktl
                                L_sl = L_all[:, kt * S + m * P:
                                             kt * S + (m + 1) * P]
                                for nq in range(4):
                                    nqs = bass.ds(nq * 512, 512)
                                    nc.tensor.matmul(ps[:, nqs], L_sl,
                                                     rh[ktl][:, nqs],
                                                     start=(ktl == 0),
                                                     stop=(ktl == 7))
                            dst = acc[m][:, jh * JH:(jh + 1) * JH]
                            if op_ == 0:
                                nc.vector.tensor_copy(dst, ps[:])
                            else:
                                nc.vector.tensor_add(dst, dst, ps[:])

            # ---------- Phase B post: cos, rmax, deg, A_norm, cosT->mtd ----
            with tc.tile_pool(name="pBP", bufs=2) as pBP, \
                 tc.tile_pool(name="pBPs", bufs=4) as pBPs, \
                 tc.tile_pool(name="pBPc", bufs=4) as pBPc, \
                 tc.tile_pool(name="psBP", bufs=4, space="PSUM") as psBP:
                for m in range(MT_):
                    am = acc[m]
                    nv = pBPs.tile([P, 1], FP, tag="nv", name="nv")
                    tmp = pBP.tile([P, P], FP, tag="diagx", name="diagx")
                    nc.vector.tensor_mul(tmp[:], ninv_sb[:, m * P:(m + 1) * P],
                                         ident[:])
                    nc.vector.reduce_sum(nv[:], tmp[:], axis=AX.X)
                    nc.vector.scalar_tensor_tensor(am[:], am[:], nv[:],
                                                   cinv_sb[:],
                                                   op0=OP.mult, op1=OP.mult)
                    rmax = rvec[:, m * 4 + 0: m * 4 + 1]
                    rinv = rvec[:, m * 4 + 1: m * 4 + 2]
                    dinv = rvec[:, m * 4 + 2: m * 4 + 3]
                    cm = rvec[:, m * 4 + 3: m * 4 + 4]
                    rsum = pBPs.tile([P, 1], FP, tag="rsum", name="rsum")
                    nc.vector.reduce_max(rmax, am[:], axis=AX.X)
                    nc.vector.reduce_sum(rsum[:], am[:], axis=AX.X)
                    nc.vector.reciprocal(rinv, rmax)
                    an = pBP.tile([P, N], FP, tag="an", name="an")
                    nc.vector.tensor_scalar_mul(an[:], am[:], rinv)
                    nc.sync.dma_start(anorm[m * P:(m + 1) * P, :], an[:])
                    dg = pBPs.tile([P, 1], FP, tag="dg", name="dg")
                    nc.vector.tensor_scalar_add(dg[:], rsum[:], -1.0)
                    nc.vector.tensor_mul(dg[:], dg[:], rinv)
                    sd = pBPs.tile([P, 1], FP, tag="sd", name="sd")
                    nc.vector.tensor_scalar_max(sd[:], dg[:], 1e-30)
                    nc.scalar.sqrt(sd[:], sd[:])
                    di_t = pBPs.tile([P, 1], FP, tag="di_t", name="di_t")
                    nc.vector.reciprocal(di_t[:], sd[:])
                    msk = pBPs.tile([P, 1], FP, tag="msk", name="msk")
                    nc.vector.tensor_scalar(msk[:], dg[:], 0.0, None,
                                            op0=OP.is_gt)
                    nc.vector.tensor_mul(dinv, di_t[:], msk[:])
                    cm2 = pBPs.tile([P, 1], FP, tag="cm2", name="cm2")
                    nc.vector.tensor_mul(cm2[:], dinv, dinv)
                    nc.vector.tensor_mul(cm, cm2[:], rinv)
                    nc.sync.dma_start(cc1i[0:1, m * P:(m + 1) * P], rmax)
                    nc.sync.dma_start(cc1i[1:2, m * P:(m + 1) * P], dinv)
                    # cosT -> mtd (bf16), diag untouched (corrected later)
                    for jt in range(NT):
                        pt = psBP.tile([P, P], FP, tag="ptBP", name="ptBP")
                        nc.tensor.transpose(pt[:], am[:, jt * P:(jt + 1) * P],
                                            ident[:])
                        ct = pBPc.tile([P, P], BF, tag="ctBP", name="ctBP")
                        nc.vector.tensor_copy(ct[:], pt[:])
                        nc.sync.dma_start(
                            mtd[jt * P:(jt + 1) * P, m * P:(m + 1) * P],
                            ct[:])

    if BUILD_STAGE == "B":
        return
    # =============================================================
    # Collective 1, then Phase C
    # =============================================================
    if USE_COLLECTIVES and NUM_CC >= 1:
        nc.gpsimd.collective_compute(
            "AllGather", OP.bypass, replica_groups=[list(range(NCORE))],
            ins=[cc1i.ap()], outs=[cc1o.ap()])

    with tc.tile_pool(name="persC", bufs=1) as persC, \
         tc.tile_pool(name="pC", bufs=3) as pC, \
         tc.tile_pool(name="psC", bufs=2, space="PSUM") as psC, \
         tc.tile_pool(name="psCt", bufs=1, space="PSUM") as psCt, \
         tc.tile_pool(name="psCp", bufs=2, space="PSUM") as psCp, \
         tc.tile_pool(name="psCs", bufs=1, space="PSUM") as psCs:

        mt_all = persC.tile([P, NT * S], BF, tag="mt_all", name="mt_all")
        xs_all = persC.tile([P, MT_ * F], FP, tag="xs_all", name="xs_all")
        xb_all = persC.tile([P, NT * F], BF, tag="xb_all", name="xb_all")
        txg_all = persC.tile([P, NT * F], BF, tag="txg_all", name="txg_all")
        tx1_all = persC.tile([P, MT_ * F], FP, tag="tx1_all", name="tx1_all")
        tx2_all = persC.tile([P, MT_ * F], FP, tag="tx2_all", name="tx2_all")
        xsT_a = persC.tile([P, S], FP, tag="xsT_a", name="xsT_a")
        xsT_b = persC.tile([P, S], FP, tag="xsT_b", name="xsT_b")
        t1T_a = persC.tile([P, S], FP, tag="t1T_a", name="t1T_a")
        t1T_b = persC.tile([P, S], FP, tag="t1T_b", name="t1T_b")
        t2T_a = persC.tile([P, S], FP, tag="t2T_a", name="t2T_a")
        t2T_b = persC.tile([P, S], FP, tag="t2T_b", name="t2T_b")
        w_a = persC.tile([P, 6 * O], FP, tag="w_a", name="w_a")
        w_b = persC.tile([P, 6 * O], FP, tag="w_b", name="w_b")
        s_all = persC.tile([P, 12 * O], FP, tag="s_all", name="s_all")

        # gathered per-row vectors -> wj, dsh_bc
        gat = cc1o.ap().rearrange("(r c) (q p) -> c r p q", c=2, p=P)
        rmax_g = pC.tile([P, NT], FP, tag="rmax_g", name="rmax_g")
        dinv_g = pC.tile([P, NT], FP, tag="dinv_g", name="dinv_g")
        QR = NT // NCORE
        for r in range(NCORE):
            nc.sync.dma_start(rmax_g[:, r * QR:(r + 1) * QR], gat[0, r])
            nc.sync.dma_start(dinv_g[:, r * QR:(r + 1) * QR], gat[1, r])
        rr = pC.tile([P, NT], FP, tag="rr", name="rr")
        nc.vector.reciprocal(rr[:], rmax_g[:])
        nc.vector.scalar_tensor_tensor(wj_sb[:], dinv_g[:], -1.0, rr[:],
                                       op0=OP.mult, op1=OP.mult)
        dsr = pC.tile([1, S], FP, tag="dsr", name="dsr")
        nc.sync.dma_start(dsr[:], cc1i[1:2, :])
        nc.gpsimd.partition_broadcast(dsh_bc[:], dsr[:])

        # load cosT and scale in place: MT[j,i] = cosT * wj_j * dinv_i
        for jt in range(NT):
            mt_sl = mt_all[:, jt * S:(jt + 1) * S]
            nc.sync.dma_start(mt_sl, mtd[jt * P:(jt + 1) * P, :])
            nc.vector.scalar_tensor_tensor(mt_sl, mt_sl, wj_sb[:, jt:jt + 1],
                                           dsh_bc[:],
                                           op0=OP.mult, op1=OP.mult)

        for jt in range(NT):
            st = pC.tile([P, F], FP, tag="stC", name="stC")
            nc.sync.dma_start(st[:], x[jt * P:(jt + 1) * P, :])
            nc.vector.tensor_copy(xb_all[:, jt * F:(jt + 1) * F], st[:])
        for im in range(MT_):
            nc.sync.dma_start(xs_all[:, im * F:(im + 1) * F],
                              xs[im * P:(im + 1) * P, :])
        nc.sync.dma_start(xsT_a[:], xsT[0:F0, :])
        nc.sync.dma_start(xsT_b[:F1, :], xsT[F0:F, :])
        wre = wproj.ap().rearrange("w f o -> f w o")
        nc.sync.dma_start(w_a[:], wre[0:F0])
        nc.sync.dma_start(w_b[:F1, :], wre[F0:F])

        def cheb_mm(rhs_all, im):
            ps = psC.tile([P, F], FP, tag="psC", name="psC")
            for jt in range(NT):
                nc.tensor.matmul(
                    ps[:], mt_all[:, jt * S + im * P: jt * S + (im + 1) * P],
                    rhs_all[:, jt * F:(jt + 1) * F],
                    start=(jt == 0), stop=(jt == NT - 1))
            return ps

        def transpose_to(dstA, dstB, src_sl, im):
            ptA = psCt.tile([P, P], FP, tag="ptC", name="ptC")
            nc.tensor.transpose(ptA[:], src_sl[:, 0:F0], ident[:])
            nc.vector.tensor_copy(dstA[:, im * P:(im + 1) * P], ptA[:])
            ptB = psCt.tile([P, P], FP, tag="ptC2", name="ptC2")
            nc.tensor.transpose(ptB[:F1, :], src_sl[:, F0:F], ident[:])
            nc.vector.tensor_copy(dstB[:F1, im * P:(im + 1) * P],
                                  ptB[:F1, :])

        for im in range(MT_):
            ps = cheb_mm(xb_all, im)
            cm = rvec[:, im * 4 + 3: im * 4 + 4]
            t1 = tx1_all[:, im * F:(im + 1) * F]
            nc.vector.scalar_tensor_tensor(
                t1, xs_all[:, im * F:(im + 1) * F], cm, ps[:],
                op0=OP.mult, op1=OP.add)
            nc.sync.dma_start(cc2i[im * P:(im + 1) * P, :], t1)
            transpose_to(t1T_a, t1T_b, tx1_all[:, im * F:(im + 1) * F], im)

        if USE_COLLECTIVES and NUM_CC >= 2:
            nc.gpsimd.collective_compute(
                "AllGather", OP.bypass, replica_groups=[list(range(NCORE))],
                ins=[cc2i.ap()], outs=[cc2o.ap()])
        for jt in range(NT):
            st = pC.tile([P, F], FP, tag="stC2", name="stC2")
            nc.sync.dma_start(st[:], cc2o[jt * P:(jt + 1) * P, :])
            nc.vector.tensor_copy(txg_all[:, jt * F:(jt + 1) * F], st[:])

        for im in range(MT_):
            ps = cheb_mm(txg_all, im)
            cm = rvec[:, im * 4 + 3: im * 4 + 4]
            tmp = pC.tile([P, F], FP, tag="tx2tmp", name="tx2tmp")
            nc.vector.scalar_tensor_tensor(
                tmp[:], ps[:], 2.0, xs_all[:, im * F:(im + 1) * F],
                op0=OP.mult, op1=OP.subtract)
            c2 = pC.tile([P, 1], FP, tag="c2m", name="c2m")
            nc.vector.tensor_scalar_mul(c2[:], cm, 2.0)
            t2 = tx2_all[:, im * F:(im + 1) * F]
            nc.vector.scalar_tensor_tensor(
                t2, tx1_all[:, im * F:(im + 1) * F], c2[:], tmp[:],
                op0=OP.mult, op1=OP.add)
            transpose_to(t2T_a, t2T_b, tx2_all[:, im * F:(im + 1) * F], im)

        terms = {0: [(xsT_a, xsT_b, 0)],
                 1: [(xsT_a, xsT_b, 1), (t1T_a, t1T_b, 2)],
                 2: [(xsT_a, xsT_b, 3), (t1T_a, t1T_b, 4),
                     (t2T_a, t2T_b, 5)]}
        for k in range(3):
            st_s = psCs.tile([P, O], FP, tag="st_s", name="st_s")
            st_q = psCs.tile([P, O], FP, tag="st_q", name="st_q")
            for im in range(MT_):
                ps = psCp.tile([P, O], FP, tag="psP", name="psP")
                tl = terms[k]
                nmm = 2 * len(tl)
                i = 0
                for (ta, tb, slot) in tl:
                    nc.tensor.matmul(ps[:], ta[:, im * P:(im + 1) * P],
                                     w_a[:, slot * O:(slot + 1) * O],
                                     start=(i == 0), stop=(i == nmm - 1))
                    i += 1
                    nc.tensor.matmul(ps[:], tb[:F1, im * P:(im + 1) * P],
                                     w_b[:F1, slot * O:(slot + 1) * O],
                                     start=False, stop=(i == nmm - 1))
                    i += 1
                s_sl = s_all[:, (k * MT_ + im) * O: (k * MT_ + im + 1) * O]
                nc.vector.tensor_copy(s_sl, ps[:])
                ssq = pC.tile([P, O], FP, tag="ssq", name="ssq")
                nc.vector.tensor_mul(ssq[:], s_sl, s_sl)
                nc.tensor.matmul(st_s[:], ones_f[:], s_sl,
                                 start=(im == 0), stop=(im == MT_ - 1),
                                 skip_group_check=True)
                nc.tensor.matmul(st_q[:], ones_f[:], ssq[:],
                                 start=(im == 0), stop=(im == MT_ - 1),
                                 skip_group_check=True)
            row_s = pC.tile([1, O], FP, tag="row_s", name="row_s")
            row_q = pC.tile([1, O], FP, tag="row_q", name="row_q")
            nc.vector.tensor_copy(row_s[:], st_s[0:1, :])
            nc.vector.tensor_copy(row_q[:], st_q[0:1, :])
            nc.sync.dma_start(cc3i[2 * k:2 * k + 1, :], row_s[:])
            nc.sync.dma_start(cc3i[2 * k + 1:2 * k + 2, :], row_q[:])

        if USE_COLLECTIVES and NUM_CC >= 3:
            nc.gpsimd.collective_compute(
                "AllReduce", OP.add, replica_groups=[list(range(NCORE))],
                ins=[cc3i.ap()], outs=[cc3o.ap()])

        grow = pC.tile([1, O], FP, tag="grow", name="grow")
        brow = pC.tile([1, O], FP, tag="brow", name="brow")
        nc.sync.dma_start(grow[:], gamma.ap().rearrange("(a o) -> a o", a=1))
        nc.sync.dma_start(brow[:], beta.ap().rearrange("(a o) -> a o", a=1))
        for k in range(3):
            mu = pC.tile([1, O], FP, tag="mu", name="mu")
            ms = pC.tile([1, O], FP, tag="ms", name="ms")
            nc.sync.dma_start(mu[:], cc3o[2 * k:2 * k + 1, :])
            nc.sync.dma_start(ms[:], cc3o[2 * k + 1:2 * k + 2, :])
            nc.vector.tensor_scalar_mul(mu[:], mu[:], 1.0 / N)
            nc.vector.tensor_scalar_mul(ms[:], ms[:], 1.0 / N)
            var = pC.tile([1, O], FP, tag="var", name="var")
            nc.vector.tensor_mul(var[:], mu[:], mu[:])
            nc.vector.tensor_sub(var[:], ms[:], var[:])
            nc.vector.tensor_scalar_add(var[:], var[:], EPS_BN)
            nc.scalar.sqrt(var[:], var[:])
            nc.vector.reciprocal(var[:], var[:])
            scale = pC.tile([1, O], FP, tag="scale", name="scale")
            nc.vector.tensor_mul(scale[:], var[:], grow[:])
            shift = pC.tile([1, O], FP, tag="shift", name="shift")
            nc.vector.tensor_mul(shift[:], mu[:], scale[:])
            nc.vector.tensor_sub(shift[:], brow[:], shift[:])
            sc_bc = pC.tile([P, O], FP, tag="sc_bc", name="sc_bc")
            sh_bc = pC.tile([P, O], FP, tag="sh_bc", name="sh_bc")
            nc.gpsimd.partition_broadcast(sc_bc[:], scale[:])
            nc.gpsimd.partition_broadcast(sh_bc[:], shift[:])
            for im in range(MT_):
                ox = pC.tile([P, O], FP, tag="ox", name="ox")
                s_sl = s_all[:, (k * MT_ + im) * O: (k * MT_ + im + 1) * O]
                nc.vector.tensor_mul(ox[:], s_sl, sc_bc[:])
                nc.vector.tensor_add(ox[:], ox[:], sh_bc[:])
                nc.sync.dma_start(xo[k][im * P:(im + 1) * P, :], ox[:])


_EXEC_CACHE = None


def _build_exec():
    """Compile once and build a cached jitted executable with shared inputs
    replicated (not 8x-concatenated) across the mesh."""
    import jax
    from jax.experimental.shard_map import shard_map
    from jax.sharding import Mesh, PartitionSpec
    from concourse import bass2jax

    nc = build_nc()
    bass2jax.install_neuronx_cc_hook()
    pname = nc.partition_id_tensor.name if nc.partition_id_tensor else None
    in_names, out_names, out_avals = [], [], []
    for alloc in nc.m.functions[0].allocations:
        if not isinstance(alloc, mybir.MemoryLocationSet):
            continue
        nm = alloc.memorylocations[0].name
        if alloc.kind == "ExternalInput":
            if nm != pname:
                in_names.append(nm)
        elif alloc.kind == "ExternalOutput":
            out_names.append(nm)
            out_avals.append(jax.core.ShapedArray(
                tuple(alloc.tensor_shape), mybir.dt.np(alloc.dtype)))
    sharded_inputs = {"xs", "xst"}     # per-core inputs; rest replicated

    def _body(*args):
        operands = list(args)
        if pname is not None:
            operands.append(bass2jax.partition_id_tensor())
        outs = bass2jax._bass_exec_p.bind(
            *operands,
            out_avals=tuple(out_avals),
            in_names=tuple(in_names + out_names
                           + ([pname] if pname else [])),
            out_names=tuple(out_names),
            lowering_input_output_aliases=(),
            sim_require_finite=True,
            sim_require_nnan=True,
            nc=nc,
        )
        return tuple(outs)

    devices = jax.devices()[:NCORE]
    mesh = Mesh(np.asarray(devices), ("core",))
    in_specs = tuple(
        PartitionSpec("core") if nm in sharded_inputs else PartitionSpec()
        for nm in in_names) + (PartitionSpec("core"),) * len(out_names)
    out_specs = (PartitionSpec("core"),) * len(out_names)
    fn = jax.jit(shard_map(_body, mesh=mesh, in_specs=in_specs,
                           out_specs=out_specs, check_rep=False),
                 keep_unused=True)
    rep_shard = jax.sharding.NamedSharding(mesh, PartitionSpec("core"))
    zero_dev = [
        jax.device_put(
            np.zeros((NCORE * a.shape[0], *a.shape[1:]), a.dtype), rep_shard)
        for a in out_avals]
    return fn, in_names, out_names, out_avals, mesh, zero_dev


def _get_exec():
    global _EXEC_CACHE
    if _EXEC_CACHE is None:
        _EXEC_CACHE = _build_exec()
    return _EXEC_CACHE


def kernel(x, ggl_w, ggl_b, w1, b1, w2, b2, w3, b3, gamma, beta):
    import jax

    x = np.ascontiguousarray(np.asarray(x, np.float32))
    ggl_w = np.ascontiguousarray(np.asarray(ggl_w, np.float32))
    ggl_b = np.ascontiguousarray(np.asarray(ggl_b, np.float32))
    wproj = np.ascontiguousarray(np.stack([
        np.asarray(w1[0]), np.asarray(w2[0]), np.asarray(w2[1]),
        np.asarray(w3[0]), np.asarray(w3[1]), np.asarray(w3[2])
    ]).astype(np.float32))
    gamma = np.ascontiguousarray(np.asarray(gamma, np.float32))
    beta = np.ascontiguousarray(np.asarray(beta, np.float32))

    fn, in_names, out_names, out_avals, mesh, zero_dev = _get_exec()
    vals = {
        "x": x, "xt": np.ascontiguousarray(x.T),
        "xs": x,                                   # [N,F] = concat of shards
        "xst": np.ascontiguousarray(
            x.reshape(NCORE, S, F).transpose(0, 2, 1).reshape(NCORE * F, S)),
        "gw": ggl_w, "gb": ggl_b, "wproj": wproj,
        "gamma": gamma, "beta": beta,
    }
    args = [vals[nm] for nm in in_names] + list(zero_dev)
    outs = fn(*args)
    res = {nm: np.asarray(o) for nm, o in zip(out_names, outs)}
    A_norm = res["anorm"].reshape(N, N)
    xs_out = [res[f"xo{k}"].reshape(N, O) for k in range(3)]
    return (xs_out[0], xs_out[1], xs_out[2], A_norm)
